# revision 5
# baseline (speedup 1.0000x reference)
"""GraphUNet (3-level top-k pool + SAGE convs) on 8 Trainium2 NeuronCores.

Strategy (graph/data parallel, per the sharding hint):
  * Everything is kept in the ORIGINAL 65536-node space; pooled-out nodes are
    zero rows. This makes the edge structure static: no per-level relabeling.
  * Edges are sorted by dst and sharded across the 8 cores by dst block
    (core c owns rows [c*8192, (c+1)*8192)). Each core runs the same NEFF:
    indirect-DMA gather of src feature rows from a replicated [N+1, 132]
    table (features + edge-weight column), segment-sum via one-hot matmuls
    into PSUM per 128-dst group, then mean @ Wl + X_own @ Wr on TensorE.
  * Between launches the host does the cheap O(N)/O(E) glue: row norms,
    stable top-k ranks, tanh gating, relu/masking, loss reductions, the
    full-array relay that replaces an on-device all-gather, and per-level
    EDGE COMPACTION: only edges whose src matters for this level's degree /
    features are shipped, which cuts the indirect-gather count ~4.5x (the
    gather is SWDGE-descriptor-bound at ~1.4us per 128 rows). The final
    conv's degree counts ALL edges, so the dropped edges' degree
    contribution is supplied as a per-node DBASE input.

Self-contained: hardcodes N=65536, E=2^20, C=128, 8 cores.
"""

import sys

for _p in ("/opt/trn_rl_repo",):
    if _p not in sys.path:
        sys.path.insert(0, _p)

from contextlib import ExitStack

import numpy as np

import concourse.bass as bass
import concourse.bacc as bacc
import concourse.mybir as mybir
from concourse import bass_utils
from concourse.masks import make_identity
from concourse.tile import TileContext

N, E, C = 65536, 1_048_576, 128
DEPTH, RATIO = 3, 0.5
NCORES = 8
OWN = N // NCORES  # 8192
GPC = OWN // 128  # 64 dst groups per core
W = 132  # gathered row width: 128 feat + weight + 3 pad
F32 = mybir.dt.float32
I32 = mybir.dt.int32

# ---------------------------------------------------------------------------
# per-level edge metadata (edges pre-sorted by dst once; subsets per level)
# ---------------------------------------------------------------------------


class _Edges:
    def __init__(self, edge_index):
        src = edge_index[0].astype(np.int64)
        dst = edge_index[1].astype(np.int64)
        order = np.argsort(dst, kind="stable")
        self.src = src[order]
        self.dst = dst[order]


def _build_slots(src_sub, dst_sub):
    """Pack an edge subset (sorted by dst) into [8, 128, NT] slot arrays."""
    NGROUP = NCORES * GPC
    g_of_edge = dst_sub // 128
    gcounts = np.bincount(g_of_edge, minlength=NGROUP)
    Tg = max(int(np.ceil(gcounts.max() / 128)), 1) if len(src_sub) else 1
    NT = GPC * Tg

    src_all = np.full((NCORES, 128, NT), N, dtype=np.int32)
    seg_all = np.zeros((NCORES, 128, NT), dtype=np.float32)
    if len(src_sub):
        gstart = np.concatenate([[0], np.cumsum(gcounts)])
        e_rank = np.arange(len(src_sub)) - gstart[g_of_edge]
        t = e_rank // 128
        p = e_rank % 128
        gi_local = (g_of_edge * Tg + t) - (g_of_edge // GPC) * NT
        core = (g_of_edge // GPC).astype(np.int64)
        src_all[core, p, gi_local] = src_sub
        seg_all[core, p, gi_local] = (dst_sub - g_of_edge * 128).astype(np.float32)
    return Tg, NT, src_all, seg_all


# ---------------------------------------------------------------------------
# device kernel (one masked-mean SAGE conv, dst-sharded, original space)
# ---------------------------------------------------------------------------


def _build_nc(NT, Tg):
    nc = bacc.Bacc("TRN2", target_bir_lowering=False, debug=False, num_devices=NCORES)
    io = {}
    io["Y"] = nc.dram_tensor("Y", [N + 1, W], F32, kind="ExternalInput")
    io["XPT"] = nc.dram_tensor("XPT", [C, OWN], F32, kind="ExternalInput")
    io["Wl"] = nc.dram_tensor("Wl", [C, C], F32, kind="ExternalInput")
    io["Wr"] = nc.dram_tensor("Wr", [C, C], F32, kind="ExternalInput")
    io["SRC"] = nc.dram_tensor("SRC", [C, NT], I32, kind="ExternalInput")
    io["SEG"] = nc.dram_tensor("SEG", [C, NT], F32, kind="ExternalInput")
    io["DBASE"] = nc.dram_tensor("DBASE", [C, GPC], F32, kind="ExternalInput")
    io["HOUT"] = nc.dram_tensor("HOUT", [OWN, C], F32, kind="ExternalOutput")

    with TileContext(nc) as tc:
        with ExitStack() as ctx:
            const_pool = ctx.enter_context(tc.tile_pool(name="const", bufs=1))
            xw_pool = ctx.enter_context(tc.tile_pool(name="xw", bufs=8))
            s_pool = ctx.enter_context(tc.tile_pool(name="sp", bufs=6))
            ep_pool = ctx.enter_context(tc.tile_pool(name="ep", bufs=2))
            out_pool = ctx.enter_context(tc.tile_pool(name="hout", bufs=2))
            psum_u = ctx.enter_context(tc.tile_pool(name="psU", bufs=2, space="PSUM"))
            psum_t = ctx.enter_context(tc.tile_pool(name="psT", bufs=2, space="PSUM"))
            psum_h = ctx.enter_context(tc.tile_pool(name="psH", bufs=2, space="PSUM"))

            src_res = const_pool.tile([C, NT], I32, tag="src")
            seg_res = const_pool.tile([C, NT], F32, tag="seg")
            dbase_res = const_pool.tile([C, GPC], F32, tag="dbase")
            xpt_res = const_pool.tile([C, OWN], F32, tag="xpt")
            wl_t = const_pool.tile([C, C], F32, tag="wl")
            wr_t = const_pool.tile([C, C], F32, tag="wr")
            ident = const_pool.tile([C, C], F32, tag="ident")
            iota_t = const_pool.tile([C, C], F32, tag="iota")

            nc.sync.dma_start(out=src_res[:], in_=io["SRC"][:])
            nc.sync.dma_start(out=seg_res[:], in_=io["SEG"][:])
            nc.sync.dma_start(out=dbase_res[:], in_=io["DBASE"][:])
            nc.sync.dma_start(out=xpt_res[:], in_=io["XPT"][:])
            nc.sync.dma_start(out=wl_t[:], in_=io["Wl"][:])
            nc.sync.dma_start(out=wr_t[:], in_=io["Wr"][:])
            make_identity(nc, ident[:])
            nc.gpsimd.iota(
                iota_t[:],
                pattern=[[1, C]],
                base=0,
                channel_multiplier=0,
                allow_small_or_imprecise_dtypes=True,
            )

            U = None
            for gi in range(NT):
                g, t = gi // Tg, gi % Tg
                xw = xw_pool.tile([C, W], F32, tag="xw")
                # HW indirect DMA consumes exactly one index per partition.
                nc.gpsimd.indirect_dma_start(
                    out=xw[:],
                    out_offset=None,
                    in_=io["Y"][:],
                    in_offset=bass.IndirectOffsetOnAxis(
                        ap=src_res[:, gi : gi + 1], axis=0
                    ),
                )
                if t == 0:
                    U = psum_u.tile([C, W], F32, tag="U")
                sp = s_pool.tile([C, C], F32, tag="sp")
                nc.vector.tensor_scalar(
                    out=sp[:],
                    in0=iota_t[:],
                    scalar1=seg_res[:, gi : gi + 1],
                    scalar2=xw[:, C : C + 1],
                    op0=mybir.AluOpType.is_equal,
                    op1=mybir.AluOpType.mult,
                )
                nc.tensor.matmul(
                    out=U[:],
                    lhsT=sp[:],
                    rhs=xw[:, 0:W],
                    start=(t == 0),
                    stop=(t == Tg - 1),
                )
                if t == Tg - 1:
                    dc = ep_pool.tile([C, 1], F32, tag="dc")
                    rc = ep_pool.tile([C, 1], F32, tag="rc")
                    mean = ep_pool.tile([C, C], F32, tag="mean")
                    # deg = max(U[:,128] + dbase[:,g], 1)
                    nc.vector.tensor_scalar(
                        out=dc[:],
                        in0=U[:, C : C + 1],
                        scalar1=dbase_res[:, g : g + 1],
                        scalar2=1.0,
                        op0=mybir.AluOpType.add,
                        op1=mybir.AluOpType.max,
                    )
                    nc.vector.reciprocal(rc[:], dc[:])
                    nc.vector.tensor_scalar(
                        out=mean[:],
                        in0=U[:, 0:C],
                        scalar1=rc[:, 0:1],
                        scalar2=None,
                        op0=mybir.AluOpType.mult,
                    )
                    meant_p = psum_t.tile([C, C], F32, tag="meantp")
                    nc.tensor.transpose(meant_p[:], mean[:], ident[:])
                    meant = ep_pool.tile([C, C], F32, tag="meant")
                    nc.scalar.copy(meant[:], meant_p[:])
                    hp = psum_h.tile([C, C], F32, tag="hp")
                    nc.tensor.matmul(
                        hp[:], lhsT=meant[:], rhs=wl_t[:], start=True, stop=False
                    )
                    nc.tensor.matmul(
                        hp[:],
                        lhsT=xpt_res[:, g * C : (g + 1) * C],
                        rhs=wr_t[:],
                        start=False,
                        stop=True,
                    )
                    hs = out_pool.tile([C, C], F32, tag="hs")
                    nc.scalar.copy(hs[:], hp[:])
                    nc.sync.dma_start(
                        out=io["HOUT"][g * C : (g + 1) * C, :], in_=hs[:]
                    )
    nc.finalize()
    return nc


# ---------------------------------------------------------------------------
# host orchestration
# ---------------------------------------------------------------------------


def _stable_topk_desc(s, k):
    order = np.argsort(-s, kind="stable")
    perm = order[:k]
    return s[perm], perm


class _DeviceConv:
    def __init__(self, edge_index):
        self.ed = _Edges(edge_index)
        self.nc_cache = {}
        self.launches = 0
        self.trace = False
        self.results = []

    def __call__(self, Y_feat, degmask, srcfilter, Wl_i, Wr_i):
        """One conv launch.

        degmask: per-node edge weight (the Y presence column); deg(v) must
                 equal sum of degmask(u) over ALL original in-edges u->v.
        srcfilter: only edges with srcfilter[src]=1 are shipped; dropped
                 edges' degree contribution goes in via DBASE.
        """
        ed = self.ed
        keep = srcfilter[ed.src] > 0.5
        src_sub = ed.src[keep]
        dst_sub = ed.dst[keep]
        dbase_full = np.bincount(
            ed.dst, weights=degmask[ed.src], minlength=N
        ) - np.bincount(dst_sub, weights=degmask[src_sub], minlength=N)
        dbase_full = dbase_full.astype(np.float32)

        Tg, NT, src_all, seg_all = _build_slots(src_sub, dst_sub)
        key = (NT, Tg)
        if key not in self.nc_cache:
            self.nc_cache[key] = _build_nc(NT, Tg)
        nc = self.nc_cache[key]

        Y = np.zeros((N + 1, W), np.float32)
        Y[:N, :C] = Y_feat
        Y[:N, C] = degmask
        # DBASE layout per core: [row-within-group 128, group GPC]
        dbase_l = dbase_full.reshape(NCORES, GPC, 128).transpose(0, 2, 1).copy()
        in_maps = []
        for c in range(NCORES):
            in_maps.append(
                {
                    "Y": Y,
                    "XPT": np.ascontiguousarray(Y_feat[c * OWN : (c + 1) * OWN].T),
                    "Wl": np.ascontiguousarray(Wl_i, dtype=np.float32),
                    "Wr": np.ascontiguousarray(Wr_i, dtype=np.float32),
                    "SRC": src_all[c],
                    "SEG": seg_all[c],
                    "DBASE": dbase_l[c],
                }
            )
        res = bass_utils.run_bass_kernel_spmd(
            nc, in_maps, core_ids=list(range(NCORES)), trace=self.trace
        )
        self.launches += 1
        self.results.append((key, res))
        return np.concatenate(
            [res.results[c]["HOUT"] for c in range(NCORES)], axis=0
        ).astype(np.float32)


def _host_forward(x, edge_index, Wl, Wr, b, conv):
    x0 = np.asarray(x, np.float32)
    L = np.float32(0.0)

    lab2orig = np.arange(N, dtype=np.int64)
    F_lab = x0
    down_lab2orig = [lab2orig]
    down_alive = [np.ones(N, np.float32)]
    down_h_lab = [F_lab]

    n = N
    for i in range(DEPTH):
        k = int(np.ceil(RATIO * n))
        s_lab = np.sqrt(np.sum(F_lab.astype(np.float32) ** 2, axis=-1) + 1e-12).astype(
            np.float32
        )
        vals, perm = _stable_topk_desc(s_lab, k)
        gate = np.tanh(vals).astype(np.float32)
        new_lab2orig = lab2orig[perm]
        XP_lab = (F_lab[perm] * gate[:, None]).astype(np.float32)

        XP_orig = np.zeros((N, C), np.float32)
        XP_orig[new_lab2orig] = XP_lab
        present = np.zeros(N, np.float32)
        present[new_lab2orig] = 1.0

        H_orig = conv(XP_orig, present, present, Wl[i], Wr[i]) + b[i][None, :]
        h_lab = np.maximum(H_orig[new_lab2orig], 0.0).astype(np.float32)

        if i < DEPTH - 1:
            k2 = int(np.ceil(RATIO * k))
            s2 = np.sqrt(np.sum(h_lab**2, axis=-1) + 1e-12).astype(np.float32)
            vals2, p2 = _stable_topk_desc(s2, k2)
            nx = (h_lab[p2] * np.tanh(vals2)[:, None]).astype(np.float32)
            L = L + np.float32(np.mean((h_lab[p2] - nx) ** 2, dtype=np.float32))
        L = L + np.float32(np.mean((h_lab - x0[perm]) ** 2, dtype=np.float32))
        L = L + np.float32(np.mean(np.abs(h_lab - F_lab[perm]), dtype=np.float32))

        lab2orig = new_lab2orig
        F_lab = h_lab
        down_lab2orig.append(lab2orig)
        down_alive.append(present)
        down_h_lab.append(h_lab)
        n = k

    h_lab = down_h_lab[-1]
    h_orig = np.zeros((N, C), np.float32)
    h_orig[down_lab2orig[DEPTH]] = h_lab
    for i in range(DEPTH - 1, 0, -1):
        ci = DEPTH + i - 1
        present = down_alive[i]
        H_orig = conv(h_orig, present, present, Wl[ci], Wr[ci]) + b[ci][None, :]
        h_lab = np.maximum(H_orig[down_lab2orig[i]], 0.0).astype(np.float32)
        h_orig = np.zeros((N, C), np.float32)
        h_orig[down_lab2orig[i]] = h_lab

    # final conv: degree counts ALL edges; ship only edges whose src has
    # nonzero features (alive after pool 0); the rest arrive via DBASE.
    ones = np.ones(N, np.float32)
    out = conv(h_orig, ones, down_alive[1], Wl[-1], Wr[-1]) + b[-1][None, :]
    return out.astype(np.float32), np.float32(L)


_CONV_CACHE = {}


def _get_conv(edge_index):
    key = hash(edge_index.tobytes())
    if key not in _CONV_CACHE:
        _CONV_CACHE.clear()
        _CONV_CACHE[key] = _DeviceConv(edge_index)
    return _CONV_CACHE[key]


def kernel(x, edge_index, Wl, Wr, b):
    x = np.asarray(x, np.float32)
    edge_index = np.asarray(edge_index)
    Wl = np.asarray(Wl, np.float32)
    Wr = np.asarray(Wr, np.float32)
    b = np.asarray(b, np.float32)
    conv = _get_conv(edge_index)
    out, L = _host_forward(x, edge_index, Wl, Wr, b, conv)
    return out, L


# revision 6
# speedup vs baseline: 1.4592x; 1.4592x over previous
"""GraphUNet (3-level top-k pool + SAGE convs) on 8 Trainium2 NeuronCores.

Strategy (graph/data parallel, per the sharding hint):
  * Everything is kept in the ORIGINAL 65536-node space; pooled-out nodes are
    zero rows. This makes the edge structure static: no per-level relabeling.
  * Edges are sorted by dst and sharded across the 8 cores by dst block
    (core c owns rows [c*8192, (c+1)*8192)). Each core runs the same NEFF:
    indirect-DMA gather of src feature rows from a replicated [N+1, 132]
    table (features + edge-weight column), segment-sum via one-hot matmuls
    into PSUM per 128-dst group, then mean @ Wl + X_own @ Wr on TensorE.
  * Between launches the host does the cheap O(N)/O(E) glue: row norms,
    stable top-k ranks, tanh gating, relu/masking, loss reductions, the
    full-array relay that replaces an on-device all-gather, and per-level
    EDGE COMPACTION: only edges whose src matters for this level's degree /
    features are shipped, which cuts the indirect-gather count ~4.5x (the
    gather is SWDGE-descriptor-bound at ~1.4us per 128 rows). The final
    conv's degree counts ALL edges, so the dropped edges' degree
    contribution is supplied as a per-node DBASE input.

Self-contained: hardcodes N=65536, E=2^20, C=128, 8 cores.
"""

import sys

for _p in ("/opt/trn_rl_repo",):
    if _p not in sys.path:
        sys.path.insert(0, _p)

from contextlib import ExitStack

import numpy as np

import concourse.bass as bass
import concourse.bacc as bacc
import concourse.mybir as mybir
from concourse import bass_utils
from concourse.masks import make_identity
from concourse.tile import TileContext

N, E, C = 65536, 1_048_576, 128
DEPTH, RATIO = 3, 0.5
NCORES = 8
OWN = N // NCORES  # 8192
GPC = OWN // 128  # 64 dst groups per core
W = 132  # gathered row width: 128 feat + weight + 3 pad
F32 = mybir.dt.float32
I32 = mybir.dt.int32

# ---------------------------------------------------------------------------
# per-level edge metadata (edges pre-sorted by dst once; subsets per level)
# ---------------------------------------------------------------------------


class _Edges:
    def __init__(self, edge_index):
        src = edge_index[0].astype(np.int64)
        dst = edge_index[1].astype(np.int64)
        order = np.argsort(dst, kind="stable")
        self.src = src[order]
        self.dst = dst[order]


def _build_slots(src_sub, dst_sub):
    """Pack an edge subset (sorted by dst) into [8, 128, NT] slot arrays."""
    NGROUP = NCORES * GPC
    g_of_edge = dst_sub // 128
    gcounts = np.bincount(g_of_edge, minlength=NGROUP)
    Tg = max(int(np.ceil(gcounts.max() / 128)), 1) if len(src_sub) else 1
    NT = GPC * Tg

    src_all = np.full((NCORES, 128, NT), N, dtype=np.int32)
    seg_all = np.zeros((NCORES, 128, NT), dtype=np.float32)
    if len(src_sub):
        gstart = np.concatenate([[0], np.cumsum(gcounts)])
        e_rank = np.arange(len(src_sub)) - gstart[g_of_edge]
        t = e_rank // 128
        p = e_rank % 128
        gi_local = (g_of_edge * Tg + t) - (g_of_edge // GPC) * NT
        core = (g_of_edge // GPC).astype(np.int64)
        src_all[core, p, gi_local] = src_sub
        seg_all[core, p, gi_local] = (dst_sub - g_of_edge * 128).astype(np.float32)
    return Tg, NT, src_all, seg_all


# ---------------------------------------------------------------------------
# device kernel (one masked-mean SAGE conv, dst-sharded, original space)
# ---------------------------------------------------------------------------


def _build_nc(NT, Tg):
    nc = bacc.Bacc("TRN2", target_bir_lowering=False, debug=False, num_devices=NCORES)
    io = {}
    io["Y"] = nc.dram_tensor("Y", [N + 1, W], F32, kind="ExternalInput")
    io["XPT"] = nc.dram_tensor("XPT", [C, OWN], F32, kind="ExternalInput")
    io["Wl"] = nc.dram_tensor("Wl", [C, C], F32, kind="ExternalInput")
    io["Wr"] = nc.dram_tensor("Wr", [C, C], F32, kind="ExternalInput")
    io["SRC"] = nc.dram_tensor("SRC", [C, NT], I32, kind="ExternalInput")
    io["SEG"] = nc.dram_tensor("SEG", [C, NT], F32, kind="ExternalInput")
    io["DBASE"] = nc.dram_tensor("DBASE", [C, GPC], F32, kind="ExternalInput")
    io["HOUT"] = nc.dram_tensor("HOUT", [OWN, C], F32, kind="ExternalOutput")

    with TileContext(nc) as tc:
        with ExitStack() as ctx:
            const_pool = ctx.enter_context(tc.tile_pool(name="const", bufs=1))
            xw_pool = ctx.enter_context(tc.tile_pool(name="xw", bufs=8))
            s_pool = ctx.enter_context(tc.tile_pool(name="sp", bufs=6))
            ep_pool = ctx.enter_context(tc.tile_pool(name="ep", bufs=2))
            out_pool = ctx.enter_context(tc.tile_pool(name="hout", bufs=2))
            psum_u = ctx.enter_context(tc.tile_pool(name="psU", bufs=2, space="PSUM"))
            psum_t = ctx.enter_context(tc.tile_pool(name="psT", bufs=2, space="PSUM"))
            psum_h = ctx.enter_context(tc.tile_pool(name="psH", bufs=2, space="PSUM"))

            src_res = const_pool.tile([C, NT], I32, tag="src")
            seg_res = const_pool.tile([C, NT], F32, tag="seg")
            dbase_res = const_pool.tile([C, GPC], F32, tag="dbase")
            xpt_res = const_pool.tile([C, OWN], F32, tag="xpt")
            wl_t = const_pool.tile([C, C], F32, tag="wl")
            wr_t = const_pool.tile([C, C], F32, tag="wr")
            ident = const_pool.tile([C, C], F32, tag="ident")
            iota_t = const_pool.tile([C, C], F32, tag="iota")

            nc.sync.dma_start(out=src_res[:], in_=io["SRC"][:])
            nc.sync.dma_start(out=seg_res[:], in_=io["SEG"][:])
            nc.sync.dma_start(out=dbase_res[:], in_=io["DBASE"][:])
            nc.sync.dma_start(out=xpt_res[:], in_=io["XPT"][:])
            nc.sync.dma_start(out=wl_t[:], in_=io["Wl"][:])
            nc.sync.dma_start(out=wr_t[:], in_=io["Wr"][:])
            make_identity(nc, ident[:])
            nc.gpsimd.iota(
                iota_t[:],
                pattern=[[1, C]],
                base=0,
                channel_multiplier=0,
                allow_small_or_imprecise_dtypes=True,
            )

            U = None
            for gi in range(NT):
                g, t = gi // Tg, gi % Tg
                xw = xw_pool.tile([C, W], F32, tag="xw")
                # HW indirect DMA consumes exactly one index per partition.
                nc.gpsimd.indirect_dma_start(
                    out=xw[:],
                    out_offset=None,
                    in_=io["Y"][:],
                    in_offset=bass.IndirectOffsetOnAxis(
                        ap=src_res[:, gi : gi + 1], axis=0
                    ),
                )
                if t == 0:
                    U = psum_u.tile([C, W], F32, tag="U")
                sp = s_pool.tile([C, C], F32, tag="sp")
                nc.vector.tensor_scalar(
                    out=sp[:],
                    in0=iota_t[:],
                    scalar1=seg_res[:, gi : gi + 1],
                    scalar2=xw[:, C : C + 1],
                    op0=mybir.AluOpType.is_equal,
                    op1=mybir.AluOpType.mult,
                )
                nc.tensor.matmul(
                    out=U[:],
                    lhsT=sp[:],
                    rhs=xw[:, 0:W],
                    start=(t == 0),
                    stop=(t == Tg - 1),
                )
                if t == Tg - 1:
                    dc = ep_pool.tile([C, 1], F32, tag="dc")
                    rc = ep_pool.tile([C, 1], F32, tag="rc")
                    mean = ep_pool.tile([C, C], F32, tag="mean")
                    # deg = max(U[:,128] + dbase[:,g], 1)
                    nc.vector.tensor_scalar(
                        out=dc[:],
                        in0=U[:, C : C + 1],
                        scalar1=dbase_res[:, g : g + 1],
                        scalar2=1.0,
                        op0=mybir.AluOpType.add,
                        op1=mybir.AluOpType.max,
                    )
                    nc.vector.reciprocal(rc[:], dc[:])
                    nc.vector.tensor_scalar(
                        out=mean[:],
                        in0=U[:, 0:C],
                        scalar1=rc[:, 0:1],
                        scalar2=None,
                        op0=mybir.AluOpType.mult,
                    )
                    meant_p = psum_t.tile([C, C], F32, tag="meantp")
                    nc.tensor.transpose(meant_p[:], mean[:], ident[:])
                    meant = ep_pool.tile([C, C], F32, tag="meant")
                    nc.scalar.copy(meant[:], meant_p[:])
                    hp = psum_h.tile([C, C], F32, tag="hp")
                    nc.tensor.matmul(
                        hp[:], lhsT=meant[:], rhs=wl_t[:], start=True, stop=False
                    )
                    nc.tensor.matmul(
                        hp[:],
                        lhsT=xpt_res[:, g * C : (g + 1) * C],
                        rhs=wr_t[:],
                        start=False,
                        stop=True,
                    )
                    hs = out_pool.tile([C, C], F32, tag="hs")
                    nc.scalar.copy(hs[:], hp[:])
                    nc.sync.dma_start(
                        out=io["HOUT"][g * C : (g + 1) * C, :], in_=hs[:]
                    )
    nc.finalize()
    return nc


# ---------------------------------------------------------------------------
# host orchestration
# ---------------------------------------------------------------------------


def _stable_topk_desc(s, k):
    order = np.argsort(-s, kind="stable")
    perm = order[:k]
    return s[perm], perm


class _DeviceConv:
    def __init__(self, edge_index):
        self.ed = _Edges(edge_index)
        self.nc_cache = {}
        self.launches = 0
        self.trace = False
        self.results = []

    def __call__(self, Y_feat, degmask, srcfilter, used, Wl_i, Wr_i):
        """One conv launch.

        degmask: per-node edge weight (the Y presence column); deg(v) must
                 equal sum of degmask(u) over ALL original in-edges u->v
                 for every node v with used[v]=1.
        srcfilter: only edges with srcfilter[src]=1 are shipped; dropped
                 edges' degree contribution goes in via DBASE.
        used: rows of the output that are consumed downstream; edges into
                 unused dst rows are dropped too (those rows are garbage).
        """
        ed = self.ed
        keep = (srcfilter[ed.src] > 0.5) & (used[ed.dst] > 0.5)
        src_sub = ed.src[keep]
        dst_sub = ed.dst[keep]
        dbase_full = np.bincount(
            ed.dst, weights=degmask[ed.src], minlength=N
        ) - np.bincount(dst_sub, weights=degmask[src_sub], minlength=N)
        dbase_full = dbase_full.astype(np.float32)

        Tg, NT, src_all, seg_all = _build_slots(src_sub, dst_sub)
        key = (NT, Tg)
        if key not in self.nc_cache:
            self.nc_cache[key] = _build_nc(NT, Tg)
        nc = self.nc_cache[key]

        Y = np.zeros((N + 1, W), np.float32)
        Y[:N, :C] = Y_feat
        Y[:N, C] = degmask
        # DBASE layout per core: [row-within-group 128, group GPC]
        dbase_l = dbase_full.reshape(NCORES, GPC, 128).transpose(0, 2, 1).copy()
        in_maps = []
        for c in range(NCORES):
            in_maps.append(
                {
                    "Y": Y,
                    "XPT": np.ascontiguousarray(Y_feat[c * OWN : (c + 1) * OWN].T),
                    "Wl": np.ascontiguousarray(Wl_i, dtype=np.float32),
                    "Wr": np.ascontiguousarray(Wr_i, dtype=np.float32),
                    "SRC": src_all[c],
                    "SEG": seg_all[c],
                    "DBASE": dbase_l[c],
                }
            )
        res = bass_utils.run_bass_kernel_spmd(
            nc, in_maps, core_ids=list(range(NCORES)), trace=self.trace
        )
        self.launches += 1
        self.results.append((key, res))
        return np.concatenate(
            [res.results[c]["HOUT"] for c in range(NCORES)], axis=0
        ).astype(np.float32)


def _host_forward(x, edge_index, Wl, Wr, b, conv):
    x0 = np.asarray(x, np.float32)
    L = np.float32(0.0)

    lab2orig = np.arange(N, dtype=np.int64)
    F_lab = x0
    down_lab2orig = [lab2orig]
    down_alive = [np.ones(N, np.float32)]
    down_h_lab = [F_lab]

    n = N
    for i in range(DEPTH):
        k = int(np.ceil(RATIO * n))
        s_lab = np.sqrt(np.sum(F_lab.astype(np.float32) ** 2, axis=-1) + 1e-12).astype(
            np.float32
        )
        vals, perm = _stable_topk_desc(s_lab, k)
        gate = np.tanh(vals).astype(np.float32)
        new_lab2orig = lab2orig[perm]
        XP_lab = (F_lab[perm] * gate[:, None]).astype(np.float32)

        XP_orig = np.zeros((N, C), np.float32)
        XP_orig[new_lab2orig] = XP_lab
        present = np.zeros(N, np.float32)
        present[new_lab2orig] = 1.0

        H_orig = conv(XP_orig, present, present, present, Wl[i], Wr[i]) + b[i][None, :]
        h_lab = np.maximum(H_orig[new_lab2orig], 0.0).astype(np.float32)

        if i < DEPTH - 1:
            k2 = int(np.ceil(RATIO * k))
            s2 = np.sqrt(np.sum(h_lab**2, axis=-1) + 1e-12).astype(np.float32)
            vals2, p2 = _stable_topk_desc(s2, k2)
            nx = (h_lab[p2] * np.tanh(vals2)[:, None]).astype(np.float32)
            L = L + np.float32(np.mean((h_lab[p2] - nx) ** 2, dtype=np.float32))
        L = L + np.float32(np.mean((h_lab - x0[perm]) ** 2, dtype=np.float32))
        L = L + np.float32(np.mean(np.abs(h_lab - F_lab[perm]), dtype=np.float32))

        lab2orig = new_lab2orig
        F_lab = h_lab
        down_lab2orig.append(lab2orig)
        down_alive.append(present)
        down_h_lab.append(h_lab)
        n = k

    h_lab = down_h_lab[-1]
    h_orig = np.zeros((N, C), np.float32)
    h_orig[down_lab2orig[DEPTH]] = h_lab
    for i in range(DEPTH - 1, 0, -1):
        ci = DEPTH + i - 1
        present = down_alive[i]
        H_orig = conv(h_orig, present, present, present, Wl[ci], Wr[ci]) + b[ci][None, :]
        h_lab = np.maximum(H_orig[down_lab2orig[i]], 0.0).astype(np.float32)
        h_orig = np.zeros((N, C), np.float32)
        h_orig[down_lab2orig[i]] = h_lab

    # final conv: degree counts ALL edges; ship only edges whose src has
    # nonzero features (alive after pool 0); the rest arrive via DBASE.
    ones = np.ones(N, np.float32)
    out = conv(h_orig, ones, down_alive[1], ones, Wl[-1], Wr[-1]) + b[-1][None, :]
    return out.astype(np.float32), np.float32(L)


_CONV_CACHE = {}


def _get_conv(edge_index):
    key = hash(edge_index.tobytes())
    if key not in _CONV_CACHE:
        _CONV_CACHE.clear()
        _CONV_CACHE[key] = _DeviceConv(edge_index)
    return _CONV_CACHE[key]


def kernel(x, edge_index, Wl, Wr, b):
    x = np.asarray(x, np.float32)
    edge_index = np.asarray(edge_index)
    Wl = np.asarray(Wl, np.float32)
    Wr = np.asarray(Wr, np.float32)
    b = np.asarray(b, np.float32)
    conv = _get_conv(edge_index)
    out, L = _host_forward(x, edge_index, Wl, Wr, b, conv)
    return out, L
